# revision 16
# baseline (speedup 1.0000x reference)
"""Distributed Trainium2 kernel for nn_Attndecoder_75548474737071.

Strategy: batch-shard phase 1 (decoder GRU + two attentions + gru_v +
scoring + V1) across 8 cores (16 rows each); vocab/EXT-shard phase 2
(V2 matmul + scatter + softmaxes). One AllGather moves x/renorm/p_gen
to all cores; two AllReduces combine softmax partial sums.
"""
import sys
sys.path.insert(0, "/opt/trn_rl_repo")

import numpy as np
import ml_dtypes

import concourse.bass as bass
import concourse.bacc as bacc
import concourse.mybir as mybir
import concourse.tile as tile
from concourse import library_config
from concourse.bass_utils import run_bass_kernel_spmd

BF = ml_dtypes.bfloat16
F32 = mybir.dt.float32
BF16 = mybir.dt.bfloat16
I16 = mybir.dt.int16
AF = mybir.ActivationFunctionType
OP = mybir.AluOpType

NCORE = 8
B, H, EMB = 128, 512, 256
LT, LA, L = 512, 128, 640
VOCAB, EXT = 50000, 60000
H2, H3 = 1024, 1536
BC = B // NCORE              # 16 batch rows per core
VS = VOCAB // NCORE          # 6250 vocab rows per core
VSP = 6272                   # padded vocab width (49*128)
UP = (EXT - VOCAB) // NCORE  # 1250 upper-ext cols per core
UPP = 1280                   # padded upper width
WF = VSP + UPP               # 7552 total fin width
SS = 1888                    # local_scatter sub-slice width (4*1888 = 7552)
NSS = 4
NPASS = 3
KV2 = 9                      # 1152 = 9*128 contraction rows (x | ones-row | pad)
GIN_W = H2 + L + 4           # 1668: x(1024) | v_scatter(640) | pg | pad3
DEAD_COLS = (VSP - VS) + (UPP - UP)   # 52 exp(0)=1 dead columns per core

CHUNKS_V = [(i * 512, 512) for i in range(12)] + [(6144, 128)]
CHUNKS_U = [(VSP, 512), (VSP + 512, 512), (VSP + 1024, 256)]
CHUNKS_ALL = CHUNKS_V + CHUNKS_U


def _bf(x):
    return np.ascontiguousarray(np.asarray(x, dtype=np.float32)).astype(BF)


def _f32(x):
    return np.ascontiguousarray(np.asarray(x, dtype=np.float32))


# ----------------------------------------------------------------- builder
def build_nc():
    nc = bacc.Bacc()

    def inp(name, shape, dt=F32):
        return nc.declare_dram_parameter(name, list(shape), dt, isOutput=False)

    def outp(name, shape, dt=F32):
        return nc.declare_dram_parameter(name, list(shape), dt, isOutput=True)

    ot4 = inp("ot4", [8, 4, 128, 2048], BF16)
    otn = inp("otn", [16, 4, 128, 1024], BF16)
    oa8 = inp("oa8", [8, 128, 2048], BF16)
    oan = inp("oan", [16, 128, 1024], BF16)
    xT = inp("xT", [256, BC])
    hT = inp("hT", [512, BC])
    hcfT = inp("hcfT", [512, BC])
    hcbT = inp("hcbT", [512, BC])
    maskd = inp("maskd", [BC, L])
    sidx = inp("sidx", [NPASS, NSS, 128, L], I16)
    ident = inp("ident", [128, 128])

    whtT = inp("whtT", [1024, 512], BF16)
    wstT = inp("wstT", [512, 512], BF16)
    vt = inp("vt", [512, 1], BF16)
    biast = inp("biast", [512, 1])
    whaT = inp("whaT", [1024, 512], BF16)
    wsaT = inp("wsaT", [512, 512], BF16)
    va = inp("va", [512, 1], BF16)
    biasa = inp("biasa", [512, 1])

    gd_wihT = inp("gd_wihT", [256, 1536], BF16)
    gd_whhT = inp("gd_whhT", [512, 1536], BF16)
    gd_brz = inp("gd_brz", [1024, 1])
    gd_bin = inp("gd_bin", [512, 1])
    gd_bhn = inp("gd_bhn", [512, 1])
    gv_wihT, gv_whhT, gv_brz, gv_bin, gv_bhn = {}, {}, {}, {}, {}
    for d in ("f", "b"):
        gv_wihT[d] = inp(f"gv{d}_wihT", [1024, 1536], BF16)
        gv_whhT[d] = inp(f"gv{d}_whhT", [512, 1536], BF16)
        gv_brz[d] = inp(f"gv{d}_brz", [1024, 1])
        gv_bin[d] = inp(f"gv{d}_bin", [512, 1])
        gv_bhn[d] = inp(f"gv{d}_bhn", [512, 1])

    wtT = inp("wtT", [1024, 512], BF16)
    ws2T = inp("ws2T", [512, 512], BF16)
    v2c = inp("v2c", [512, 1], BF16)
    bias2 = inp("bias2", [512, 1])
    whc = inp("whc", [1024, 1], BF16)
    wsc = inp("wsc", [512, 1], BF16)
    wxc = inp("wxc", [256, 1], BF16)
    pgb = inp("pgb", [128, 1])
    v1T = inp("v1T", [1536, 1024], BF16)
    v1b = inp("v1b", [1024, 1])
    v2T = inp("v2T", [KV2, 128, VSP], BF16)

    o_S = outp("S", [BC, 512])
    o_attn = outp("attn", [BC, L])
    o_pg = outp("pg", [128, 1])
    o_final = outp("final", [128, WF])
    o_vext = outp("vext", [128, VSP])

    g_in = nc.dram_tensor("g_in", [BC, GIN_W], F32)
    g_out = nc.dram_tensor("g_out", [128, GIN_W], F32, addr_space="Shared")
    zv_in = nc.dram_tensor("zv_in", [128, 1], F32)
    zv_out = nc.dram_tensor("zv_out", [128, 1], F32, addr_space="Shared")
    z2_in = nc.dram_tensor("z2_in", [128, 1], F32)
    z2_out = nc.dram_tensor("z2_out", [128, 1], F32, addr_space="Shared")

    RG = [list(range(NCORE))]

    tc_cm = tile.TileContext(nc)
    tc = tc_cm.__enter__()
    nc.gpsimd.load_library(library_config.local_scatter)

    res_cm = tc.tile_pool(name="res", bufs=1)
    res = res_cm.__enter__()
    ppS_cm = tc.tile_pool(name="ppS", bufs=2, space="PSUM")
    ppS = ppS_cm.__enter__()
    ppT_cm = tc.tile_pool(name="ppT", bufs=1, space="PSUM")
    ppT = ppT_cm.__enter__()
    p1w_cm = tc.tile_pool(name="p1w", bufs=1)
    p1w = p1w_cm.__enter__()
    gdw_cm = tc.tile_pool(name="gdw", bufs=1)
    gdw = gdw_cm.__enter__()
    p1s_cm = tc.tile_pool(name="p1s", bufs=1)
    p1s = p1s_cm.__enter__()

    def rtile(shape, dt, tag, src=None, pool=None):
        t = (pool or p1s).tile(shape, dt, tag=tag)
        if src is not None:
            nc.sync.dma_start(out=t[:], in_=src)
        return t

    def load_cols(w, rows, cols, dt, tag, pool=None):
        return [
            rtile([128, cols], dt, f"{tag}{k}", w[k * 128:(k + 1) * 128, :],
                  pool=pool)
            for k in range(rows // 128)
        ]

    id_sb = rtile([128, 128], F32, "ident", ident[:], pool=res)

    def pe_transpose(in_ap, p, f, tag, out_dt=F32, pool=None):
        """Transpose [p, f] -> [f, p] via PE; returns sbuf tile [f, p]."""
        ps = ppT.tile([128, 128], F32, space="PSUM", tag="pt")
        nc.tensor.transpose(out=ps[:f, :p], in_=in_ap, identity=id_sb[:p, :p])
        t = (pool or p1s).tile([f, p], out_dt, tag=tag)
        nc.scalar.activation(t[:], ps[:f, :p], AF.Copy)
        return t

    # ---------------- resident small weights ----------------
    whtT_sb = load_cols(whtT, 1024, 512, BF16, "whtT", pool=p1w)
    wstT_sb = load_cols(wstT, 512, 512, BF16, "wstT", pool=p1w)
    vt_sb = load_cols(vt, 512, 1, BF16, "vt", pool=p1w)
    biast_sb = load_cols(biast, 512, 1, F32, "biast", pool=p1w)
    whaT_sb = load_cols(whaT, 1024, 512, BF16, "whaT", pool=p1w)
    wsaT_sb = load_cols(wsaT, 512, 512, BF16, "wsaT", pool=p1w)
    va_sb = load_cols(va, 512, 1, BF16, "va", pool=p1w)
    biasa_sb = load_cols(biasa, 512, 1, F32, "biasa", pool=p1w)

    xT_f = load_cols(xT, 256, BC, F32, "xTf", pool=p1s)
    hT_f = load_cols(hT, 512, BC, F32, "hTf", pool=p1s)

    def cast_list(src, dt, tag):
        out = []
        for k, s in enumerate(src):
            t = p1s.tile(list(s.shape), dt, tag=f"{tag}{k}")
            nc.vector.tensor_copy(out=t[:], in_=s[:])
            out.append(t)
        return out

    xT_b = cast_list(xT_f, BF16, "xTb")
    hT_b = cast_list(hT_f, BF16, "hTb")

    # ---------------- GRU cell helper (T layout, [128, BC] tiles) --------
    def gru_cell(wih_sb, nki, whh_sb, brz_sb, bin_sb, bhn_sb,
                 in_b, h_b, h_f, tag):
        rz = []
        for m in range(8):
            ps = ppS.tile([128, BC], F32, space="PSUM", tag="psm")
            nmm = nki + 4
            i = 0
            for k in range(nki):
                nc.tensor.matmul(ps[:], lhsT=wih_sb[k][:, m * 128:(m + 1) * 128],
                                 rhs=in_b[k][:], start=(i == 0), stop=(i == nmm - 1))
                i += 1
            for k in range(4):
                nc.tensor.matmul(ps[:], lhsT=whh_sb[k][:, m * 128:(m + 1) * 128],
                                 rhs=h_b[k][:], start=(i == 0), stop=(i == nmm - 1))
                i += 1
            g = p1s.tile([128, BC], F32, tag=f"{tag}rz{m}")
            nc.scalar.activation(g[:], ps[:], AF.Sigmoid, bias=brz_sb[m][:])
            rz.append(g)
        out_f, out_b = [], []
        for m in range(4):
            ps_in = ppS.tile([128, BC], F32, space="PSUM", tag="psm")
            for k in range(nki):
                nc.tensor.matmul(ps_in[:], lhsT=wih_sb[k][:, (8 + m) * 128:(9 + m) * 128],
                                 rhs=in_b[k][:], start=(k == 0), stop=(k == nki - 1))
            ps_hn = ppS.tile([128, BC], F32, space="PSUM", tag="psm")
            for k in range(4):
                nc.tensor.matmul(ps_hn[:], lhsT=whh_sb[k][:, (8 + m) * 128:(9 + m) * 128],
                                 rhs=h_b[k][:], start=(k == 0), stop=(k == 3))
            hn = p1s.tile([128, BC], F32, tag=f"{tag}hn")
            nc.vector.tensor_scalar_add(hn[:], ps_hn[:], bhn_sb[m][:])
            rhn = p1s.tile([128, BC], F32, tag=f"{tag}rhn")
            nc.vector.tensor_mul(out=rhn[:], in0=rz[m][:], in1=hn[:])
            s1 = p1s.tile([128, BC], F32, tag=f"{tag}s1")
            nc.vector.tensor_add(out=s1[:], in0=ps_in[:], in1=rhn[:])
            nt = p1s.tile([128, BC], F32, tag=f"{tag}n{m}")
            nc.scalar.activation(nt[:], s1[:], AF.Tanh, bias=bin_sb[m][:])
            hmn = p1s.tile([128, BC], F32, tag=f"{tag}hmn")
            nc.vector.tensor_tensor(out=hmn[:], in0=h_f[m][:], in1=nt[:],
                                    op=OP.subtract)
            zt = p1s.tile([128, BC], F32, tag=f"{tag}zt")
            nc.vector.tensor_mul(out=zt[:], in0=rz[4 + m][:], in1=hmn[:])
            of = p1s.tile([128, BC], F32, tag=f"{tag}o{m}")
            nc.vector.tensor_add(out=of[:], in0=zt[:], in1=nt[:])
            ob = p1s.tile([128, BC], BF16, tag=f"{tag}ob{m}")
            nc.vector.tensor_copy(out=ob[:], in_=of[:])
            out_f.append(of)
            out_b.append(ob)
        return out_f, out_b

    # ---------------- decoder GRU -> S ----------------
    gd_wihT_sb = load_cols(gd_wihT, 256, 1536, BF16, "gdwih", pool=gdw)
    gd_whhT_sb = load_cols(gd_whhT, 512, 1536, BF16, "gdwhh", pool=gdw)
    gd_brz_sb = load_cols(gd_brz, 1024, 1, F32, "gdbrz", pool=gdw)
    gd_bin_sb = load_cols(gd_bin, 512, 1, F32, "gdbin", pool=gdw)
    gd_bhn_sb = load_cols(gd_bhn, 512, 1, F32, "gdbhn", pool=gdw)

    S_f, S_b = gru_cell(gd_wihT_sb, 2, gd_whhT_sb, gd_brz_sb, gd_bin_sb,
                        gd_bhn_sb, xT_b, hT_b, hT_f, "gd")

    # S natural [16, 512] -> output
    s_nat = p1s.tile([BC, 512], F32, tag="s_nat")
    for m in range(4):
        ps = ppT.tile([128, 128], F32, space="PSUM", tag="pt")
        nc.tensor.transpose(out=ps[:BC, :], in_=S_f[m][:], identity=id_sb[:])
        nc.scalar.activation(s_nat[:, m * 128:(m + 1) * 128], ps[:BC, :], AF.Copy)
    nc.sync.dma_start(out=o_S[:], in_=s_nat[:])

    # ---------------- u bias vectors for the two attentions ----------------
    def make_u(ws_sb, bias_sb, tag):
        u = []
        for m in range(4):
            ps = ppS.tile([128, BC], F32, space="PSUM", tag="psm")
            for k in range(4):
                nc.tensor.matmul(ps[:], lhsT=ws_sb[k][:, m * 128:(m + 1) * 128],
                                 rhs=S_b[k][:], start=(k == 0), stop=(k == 3))
            t = p1s.tile([128, BC], F32, tag=f"{tag}{m}")
            nc.vector.tensor_scalar_add(t[:], ps[:], bias_sb[m][:])
            u.append(t)
        return u

    u_t = make_u(wstT_sb, biast_sb, "ut")
    u_a = make_u(wsaT_sb, biasa_sb, "ua")

    # ---------------- attention over outputs_t ----------------
    pcol_cm = tc.tile_pool(name="pcol", bufs=1, space="PSUM")
    pcol_pool = pcol_cm.__enter__()
    e_t = p1s.tile([BC, 512], F32, tag="e_t")
    pcol = pcol_pool.tile([128, 64], F32, space="PSUM", tag="pcol")

    pa_cm = tc.tile_pool(name="pa", bufs=2)
    pa = pa_cm.__enter__()
    pA_cm = tc.tile_pool(name="pAt", bufs=2)
    pA = pA_cm.__enter__()
    ppB_cm = tc.tile_pool(name="ppB", bufs=2, space="PSUM")
    ppB = ppB_cm.__enter__()

    for bg in range(4):
        kt = []
        for k in range(8):
            t = pa.tile([128, 2048], BF16, tag=f"ot{k}")
            nc.sync.dma_start(out=t[:], in_=ot4[k, bg])
            kt.append(t)
        for bi in range(4):
            b = bg * 4 + bi
            A_tiles = []
            for m in range(4):
                ps = ppB.tile([128, 512], F32, space="PSUM")
                for k in range(8):
                    nc.tensor.matmul(ps[:], lhsT=whtT_sb[k][:, m * 128:(m + 1) * 128],
                                     rhs=kt[k][:, bi * 512:(bi + 1) * 512],
                                     start=(k == 0), stop=(k == 7))
                tx = pA.tile([128, 512], F32, tag=f"Tx{m}")
                nc.scalar.activation(tx[:], ps[:], AF.Exp, bias=u_t[m][:, b:b + 1])
                A = pA.tile([128, 512], BF16, tag=f"At{m}")
                nc.scalar.activation(A[:], tx[:], AF.Ln, bias=1.0)
                A_tiles.append(A)
            for lc in range(4):
                for m in range(4):
                    nc.tensor.matmul(pcol[:, lc * 16 + b:lc * 16 + b + 1],
                                     lhsT=A_tiles[m][:, lc * 128:(lc + 1) * 128],
                                     rhs=vt_sb[m][:, 0:1],
                                     start=(m == 0), stop=(m == 3))

    ecol_sb = p1s.tile([128, 64], F32, tag="ecol_sb")
    nc.scalar.activation(ecol_sb[:], pcol[:], AF.Copy)
    for lc in range(4):
        pst = ppT.tile([128, 128], F32, space="PSUM", tag="pt")
        nc.tensor.transpose(out=pst[:16, :128], in_=ecol_sb[:, lc * 16:(lc + 1) * 16],
                            identity=id_sb[:])
        nc.scalar.activation(e_t[:, lc * 128:(lc + 1) * 128], pst[:16, :128], AF.Copy)
    pA_cm.__exit__(None, None, None)
    pa_cm.__exit__(None, None, None)

    # softmax over l (no max subtraction; e is small)
    exp_t = p1s.tile([BC, 512], F32, tag="exp_t")
    se_t = p1s.tile([BC, 1], F32, tag="se_t")
    nc.scalar.activation(exp_t[:], e_t[:], AF.Exp, accum_out=se_t[:])
    rse_t = p1s.tile([BC, 1], F32, tag="rse_t")
    nc.vector.reciprocal(out=rse_t[:], in_=se_t[:])
    aw_t = p1s.tile([BC, 512], F32, tag="aw_t")
    nc.vector.tensor_scalar_mul(out=aw_t[:], in0=exp_t[:], scalar1=rse_t[:])

    # attn weights transposed (bf16) for ctx matmuls
    awT_t = [pe_transpose(aw_t[:, k * 128:(k + 1) * 128], BC, 128,
                          f"awTt{k}", BF16) for k in range(4)]

    # ctx_t via per-row matmuls into psum rows
    ctx_t_nat = p1s.tile([BC, 1024], F32, tag="ctx_t_nat")
    pctx = pcol_pool.tile([128, 128], F32, space="PSUM", tag="pctx")
    pn_cm = tc.tile_pool(name="pn", bufs=8)
    pn = pn_cm.__enter__()
    for b in range(BC):
        nts = []
        for lc in range(4):
            nt = pn.tile([128, 1024], BF16, tag="otn")
            nc.sync.dma_start(out=nt[:], in_=otn[b, lc])
            nts.append(nt)
        for fc in range(8):
            for lc in range(4):
                nc.tensor.matmul(pctx[:, fc * 16 + b:fc * 16 + b + 1],
                                 lhsT=nts[lc][:, fc * 128:(fc + 1) * 128],
                                 rhs=awT_t[lc][:, b:b + 1],
                                 start=(lc == 0), stop=(lc == 3))
    pn_cm.__exit__(None, None, None)
    ctxcol_sb = p1s.tile([128, 128], F32, tag="ctxcol_sb")
    nc.scalar.activation(ctxcol_sb[:], pctx[:], AF.Copy)
    for fc in range(8):
        pst = ppT.tile([128, 128], F32, space="PSUM", tag="pt")
        nc.tensor.transpose(out=pst[:16, :128], in_=ctxcol_sb[:, fc * 16:(fc + 1) * 16],
                            identity=id_sb[:])
        nc.scalar.activation(ctx_t_nat[:, fc * 128:(fc + 1) * 128], pst[:16, :128],
                             AF.Copy)

    # ---------------- attention over outputs_a ----------------
    pa2_cm = tc.tile_pool(name="pa2", bufs=1)
    pa2 = pa2_cm.__enter__()
    at = []
    for k in range(8):
        t = pa2.tile([128, 2048], BF16, tag=f"oa{k}")
        nc.sync.dma_start(out=t[:], in_=oa8[k])
        at.append(t)
    e_a = p1s.tile([BC, 128], F32, tag="e_a")
    pA2_cm = tc.tile_pool(name="pA2", bufs=2)
    pA2 = pA2_cm.__enter__()
    for bg in range(4):
        A_tiles = []
        for m in range(4):
            ps = ppB.tile([128, 512], F32, space="PSUM")
            for k in range(8):
                nc.tensor.matmul(ps[:], lhsT=whaT_sb[k][:, m * 128:(m + 1) * 128],
                                 rhs=at[k][:, bg * 512:(bg + 1) * 512],
                                 start=(k == 0), stop=(k == 7))
            tx = pA2.tile([128, 512], F32, tag=f"Txa{m}")
            for bi in range(4):
                b = bg * 4 + bi
                nc.scalar.activation(tx[:, bi * 128:(bi + 1) * 128],
                                     ps[:, bi * 128:(bi + 1) * 128],
                                     AF.Exp, bias=u_a[m][:, b:b + 1])
            A = pA2.tile([128, 512], BF16, tag=f"Aa{m}")
            nc.scalar.activation(A[:], tx[:], AF.Ln, bias=1.0)
            A_tiles.append(A)
        for bi in range(4):
            b = bg * 4 + bi
            for m in range(4):
                nc.tensor.matmul(pcol[:, 48 + b:48 + b + 1],
                                 lhsT=A_tiles[m][:, bi * 128:(bi + 1) * 128],
                                 rhs=va_sb[m][:, 0:1],
                                 start=(m == 0), stop=(m == 3))
    eacol_sb = p1s.tile([128, 16], F32, tag="eacol_sb")
    nc.scalar.activation(eacol_sb[:], pcol[:, 48:64], AF.Copy)
    pst_a = ppT.tile([128, 128], F32, space="PSUM", tag="pt")
    nc.tensor.transpose(out=pst_a[:16, :128], in_=eacol_sb[:], identity=id_sb[:])
    nc.scalar.activation(e_a[:], pst_a[:16, :128], AF.Copy)
    pA2_cm.__exit__(None, None, None)

    exp_a = p1s.tile([BC, 128], F32, tag="exp_a")
    se_a = p1s.tile([BC, 1], F32, tag="se_a")
    nc.scalar.activation(exp_a[:], e_a[:], AF.Exp, accum_out=se_a[:])
    rse_a = p1s.tile([BC, 1], F32, tag="rse_a")
    nc.vector.reciprocal(out=rse_a[:], in_=se_a[:])
    aw_a = p1s.tile([BC, 128], F32, tag="aw_a")
    nc.vector.tensor_scalar_mul(out=aw_a[:], in0=exp_a[:], scalar1=rse_a[:])
    awT_a = pe_transpose(aw_a[:], BC, 128, "awTa", BF16)

    ctx_a_nat = p1s.tile([BC, 1024], F32, tag="ctx_a_nat")
    pctxa = pcol_pool.tile([128, 128], F32, space="PSUM", tag="pctx")
    pn2_cm = tc.tile_pool(name="pn2", bufs=3)
    pn2 = pn2_cm.__enter__()
    for b in range(BC):
        nt = pn2.tile([128, 1024], BF16, tag="oan")
        nc.sync.dma_start(out=nt[:], in_=oan[b])
        for fc in range(8):
            nc.tensor.matmul(pctxa[:, fc * 16 + b:fc * 16 + b + 1],
                             lhsT=nt[:, fc * 128:(fc + 1) * 128],
                             rhs=awT_a[:, b:b + 1], start=True, stop=True)
    pn2_cm.__exit__(None, None, None)
    pa2_cm.__exit__(None, None, None)
    ctxacol_sb = p1s.tile([128, 128], F32, tag="ctxacol_sb")
    nc.scalar.activation(ctxacol_sb[:], pctxa[:], AF.Copy)
    for fc in range(8):
        pst = ppT.tile([128, 128], F32, space="PSUM", tag="pt")
        nc.tensor.transpose(out=pst[:16, :128], in_=ctxacol_sb[:, fc * 16:(fc + 1) * 16],
                            identity=id_sb[:])
        nc.scalar.activation(ctx_a_nat[:, fc * 128:(fc + 1) * 128], pst[:16, :128],
                             AF.Copy)

    # ---------------- ctx transposed (bf16 + f32) ----------------
    def t_cols(nat, tag):
        bfl, f32l = [], []
        for k in range(8):
            ps = ppT.tile([128, 128], F32, space="PSUM", tag="pt")
            nc.tensor.transpose(out=ps[:, :BC], in_=nat[:, k * 128:(k + 1) * 128],
                                identity=id_sb[:BC, :BC])
            tb = p1s.tile([128, BC], BF16, tag=f"{tag}b{k}")
            nc.scalar.activation(tb[:], ps[:, :BC], AF.Copy)
            tf = p1s.tile([128, BC], F32, tag=f"{tag}f{k}")
            nc.scalar.activation(tf[:], ps[:, :BC], AF.Copy)
            bfl.append(tb)
            f32l.append(tf)
        return bfl, f32l

    ctT_b, ctT_f = t_cols(ctx_t_nat, "ctT")
    caT_b, caT_f = t_cols(ctx_a_nat, "caT")

    # ---------------- gru_v (bidirectional over the 2 contexts) ----------
    hcf_f = load_cols(hcfT, 512, BC, F32, "hcff")
    hcb_f = load_cols(hcbT, 512, BC, F32, "hcbf")
    hcf_b = cast_list(hcf_f, BF16, "hcfb")
    hcb_b = cast_list(hcb_f, BF16, "hcbb")

    gvw_cm = tc.tile_pool(name="gvw", bufs=1)
    gvw = gvw_cm.__enter__()
    gv_sb = {}
    for d in ("f", "b"):
        gv_sb[d] = dict(
            wih=load_cols(gv_wihT[d], 1024, 1536, BF16, f"gv{d}wih", pool=gvw),
            whh=load_cols(gv_whhT[d], 512, 1536, BF16, f"gv{d}whh", pool=gvw),
            brz=load_cols(gv_brz[d], 1024, 1, F32, f"gv{d}brz", pool=gvw),
            bin=load_cols(gv_bin[d], 512, 1, F32, f"gv{d}bin", pool=gvw),
            bhn=load_cols(gv_bhn[d], 512, 1, F32, f"gv{d}bhn", pool=gvw),
        )

    def vcell(d, in_b, h_b, h_f, tag):
        w = gv_sb[d]
        return gru_cell(w["wih"], 8, w["whh"], w["brz"], w["bin"], w["bhn"],
                        in_b, h_b, h_f, tag)

    h1_f, h1_b = vcell("f", ctT_b, hcf_b, hcf_f, "h1")
    h2_f, h2_b = vcell("f", caT_b, h1_b, h1_f, "h2")
    g1_f, g1_b = vcell("b", caT_b, hcb_b, hcb_f, "g1")
    g2_f, g2_b = vcell("b", ctT_b, g1_b, g1_f, "g2")

    gvw_cm.__exit__(None, None, None)
    rep0_b, rep0_f = h1_b + g2_b, h1_f + g2_f
    rep1_b, rep1_f = h2_b + g1_b, h2_f + g1_f

    # ---------------- scoring -> weights [16, 2] ----------------
    scw_cm = tc.tile_pool(name="scw", bufs=1)
    scw = scw_cm.__enter__()
    wtT_sb = load_cols(wtT, 1024, 512, BF16, "wtT", pool=scw)
    ws2T_sb = load_cols(ws2T, 512, 512, BF16, "ws2T", pool=scw)
    v2c_sb = load_cols(v2c, 512, 1, BF16, "v2c", pool=scw)
    bias2_sb = load_cols(bias2, 512, 1, F32, "bias2", pool=scw)

    u2 = []
    for m in range(4):
        ps = ppS.tile([128, BC], F32, space="PSUM", tag="psm")
        for k in range(4):
            nc.tensor.matmul(ps[:], lhsT=ws2T_sb[k][:, m * 128:(m + 1) * 128],
                             rhs=S_b[k][:], start=(k == 0), stop=(k == 3))
        t = p1s.tile([128, BC], F32, tag=f"u2{m}")
        nc.scalar.activation(t[:], ps[:], AF.Copy)
        u2.append(t)

    pe2col = pcol_pool.tile([BC, 2], F32, space="PSUM", tag="pctx")
    for r, rep in enumerate((rep0_b, rep1_b)):
        A2s = []
        for m in range(4):
            ps = ppS.tile([128, BC], F32, space="PSUM", tag="psm")
            for k in range(8):
                nc.tensor.matmul(ps[:], lhsT=wtT_sb[k][:, m * 128:(m + 1) * 128],
                                 rhs=rep[k][:], start=(k == 0), stop=(k == 7))
            s1 = p1s.tile([128, BC], F32, tag="sc_s1")
            nc.vector.tensor_add(out=s1[:], in0=ps[:], in1=u2[m][:])
            A2 = p1s.tile([128, BC], BF16, tag=f"sc_A2{m}")
            nc.scalar.activation(A2[:], s1[:], AF.Tanh, bias=bias2_sb[m][:])
            A2s.append(A2)
        for m in range(4):
            nc.tensor.matmul(pe2col[:, r:r + 1], lhsT=A2s[m][:],
                             rhs=v2c_sb[m][:, 0:1],
                             start=(m == 0), stop=(m == 3))

    e2n = p1s.tile([BC, 2], F32, tag="e2n")
    nc.vector.tensor_copy(out=e2n[:], in_=pe2col[:])
    exp2 = p1s.tile([BC, 2], F32, tag="exp2")
    sw = p1s.tile([BC, 1], F32, tag="sw")
    nc.scalar.activation(exp2[:], e2n[:], AF.Exp, accum_out=sw[:])
    rsw = p1s.tile([BC, 1], F32, tag="rsw")
    nc.vector.reciprocal(out=rsw[:], in_=sw[:])
    w_nat = p1s.tile([BC, 2], F32, tag="w_nat")
    nc.vector.tensor_scalar_mul(out=w_nat[:], in0=exp2[:], scalar1=rsw[:])

    # broadcast weights to [128, BC] via ones-matmul
    ones_row = p1s.tile([1, 128], F32, tag="ones_row")
    nc.vector.memset(ones_row[:], 1.0)
    wb_f = []
    for r in range(2):
        wr = pe_transpose(w_nat[:, r:r + 1], BC, 1, f"wTr{r}", F32)  # [1, BC]
        ps = ppT.tile([128, 128], F32, space="PSUM", tag="pt")
        nc.tensor.matmul(ps[:, :BC], lhsT=ones_row[:], rhs=wr[:],
                         start=True, stop=True)
        t = p1s.tile([128, BC], F32, tag=f"wb{r}")
        nc.scalar.activation(t[:], ps[:, :BC], AF.Copy)
        wb_f.append(t)

    # context (T layout)
    cxT_b, cxT_f = [], []
    for k in range(8):
        t0 = p1s.tile([128, BC], F32, tag="cx_t0")
        nc.vector.tensor_mul(out=t0[:], in0=rep0_f[k][:], in1=wb_f[0][:])
        t1 = p1s.tile([128, BC], F32, tag="cx_t1")
        nc.vector.tensor_mul(out=t1[:], in0=rep1_f[k][:], in1=wb_f[1][:])
        tf = p1s.tile([128, BC], F32, tag=f"cxf{k}")
        nc.vector.tensor_add(out=tf[:], in0=t0[:], in1=t1[:])
        tb = p1s.tile([128, BC], BF16, tag=f"cxb{k}")
        nc.vector.tensor_copy(out=tb[:], in_=tf[:])
        cxT_f.append(tf)
        cxT_b.append(tb)

    # ---------------- p_gen ----------------
    whc_sb = load_cols(whc, 1024, 1, BF16, "whc")
    wsc_sb = load_cols(wsc, 512, 1, BF16, "wsc")
    wxc_sb = load_cols(wxc, 256, 1, BF16, "wxc")
    pgb_sb = rtile([128, 1], F32, "pgb", pgb[:])

    ppgc = pcol_pool.tile([BC, 1], F32, space="PSUM", tag="pctx")
    nmm = 8 + 4 + 2
    i = 0
    for k in range(8):
        nc.tensor.matmul(ppgc[:], lhsT=cxT_b[k][:], rhs=whc_sb[k][:, 0:1],
                         start=(i == 0), stop=(i == nmm - 1)); i += 1
    for k in range(4):
        nc.tensor.matmul(ppgc[:], lhsT=S_b[k][:], rhs=wsc_sb[k][:, 0:1],
                         start=(i == 0), stop=(i == nmm - 1)); i += 1
    for k in range(2):
        nc.tensor.matmul(ppgc[:], lhsT=xT_b[k][:], rhs=wxc_sb[k][:, 0:1],
                         start=(i == 0), stop=(i == nmm - 1)); i += 1
    pg_nat = p1s.tile([BC, 2], F32, tag="pg_nat")  # col0 pg, col1 1-pg
    nc.scalar.activation(pg_nat[:, 0:1], ppgc[:], AF.Sigmoid, bias=pgb_sb[:BC, :])
    nc.scalar.activation(pg_nat[:, 1:2], pg_nat[:, 0:1], AF.Copy,
                         scale=-1.0, bias=1.0)

    # ---------------- attn_dist, renorm, v_scatter ----------------
    attnd = p1s.tile([BC, L], F32, tag="attnd")
    nc.vector.tensor_scalar_mul(out=attnd[:, 0:512], in0=aw_t[:],
                                scalar1=w_nat[:, 0:1])
    nc.vector.tensor_scalar_mul(out=attnd[:, 512:640], in0=aw_a[:],
                                scalar1=w_nat[:, 1:2])
    nc.sync.dma_start(out=o_attn[:], in_=attnd[:])

    mask_sb = rtile([BC, L], F32, "mask", maskd[:])
    masked = p1s.tile([BC, L], F32, tag="masked")
    nc.vector.tensor_mul(out=masked[:], in0=attnd[:], in1=mask_sb[:])
    msum = p1s.tile([BC, 1], F32, tag="msum")
    nc.vector.reduce_sum(out=msum[:], in_=masked[:], axis=mybir.AxisListType.X)
    rmsum = p1s.tile([BC, 1], F32, tag="rmsum")
    nc.vector.reciprocal(out=rmsum[:], in_=msum[:])
    renorm = p1s.tile([BC, L], F32, tag="renorm")
    nc.vector.tensor_scalar_mul(out=renorm[:], in0=masked[:], scalar1=rmsum[:])
    v_sc = p1s.tile([BC, L], F32, tag="v_sc")
    nc.vector.tensor_scalar_mul(out=v_sc[:], in0=renorm[:],
                                scalar1=pg_nat[:, 1:2])

    # ---------------- V1 -> x, assemble gather block ----------------
    scw_cm.__exit__(None, None, None)
    v1w_cm = tc.tile_pool(name="v1w", bufs=1)
    v1w = v1w_cm.__enter__()
    v1T_sb = load_cols(v1T, 1536, 1024, BF16, "v1T", pool=v1w)
    v1b_sb = load_cols(v1b, 1024, 1, F32, "v1b", pool=v1w)
    catT = S_b + cxT_b  # 12 tiles [128, BC] bf16

    gin = p1s.tile([BC, GIN_W], F32, tag="gin")
    for m in range(8):
        ps = ppS.tile([128, BC], F32, space="PSUM", tag="psm")
        for k in range(12):
            nc.tensor.matmul(ps[:], lhsT=v1T_sb[k][:, m * 128:(m + 1) * 128],
                             rhs=catT[k][:], start=(k == 0), stop=(k == 11))
        xm = p1s.tile([128, BC], F32, tag="xm")
        nc.vector.tensor_scalar_add(xm[:], ps[:], v1b_sb[m][:])
        pst = ppT.tile([128, 128], F32, space="PSUM", tag="pt")
        nc.tensor.transpose(out=pst[:BC, :], in_=xm[:], identity=id_sb[:])
        nc.scalar.activation(gin[:, m * 128:(m + 1) * 128], pst[:BC, :], AF.Copy)
    nc.vector.tensor_copy(out=gin[:, H2:H2 + L], in_=v_sc[:])
    nc.vector.tensor_copy(out=gin[:, H2 + L:H2 + L + 1], in_=pg_nat[:, 0:1])
    nc.vector.memset(gin[:, H2 + L + 1:GIN_W], 0.0)

    nc.sync.dma_start(out=g_in[:], in_=gin[:])
    nc.gpsimd.collective_compute("AllGather", OP.bypass, replica_groups=RG,
                                 ins=[g_in[:]], outs=[g_out[:]])
    v1w_cm.__exit__(None, None, None)
    p1s_cm.__exit__(None, None, None)
    gdw_cm.__exit__(None, None, None)
    p1w_cm.__exit__(None, None, None)
    p2s_cm = tc.tile_pool(name="p2s", bufs=1)
    p2s = p2s_cm.__enter__()
    gath = p2s.tile([128, GIN_W], F32, tag="gath")
    nc.sync.dma_start(out=gath[:], in_=g_out[:])

    # ---------------- phase 2: xgT, scatter, V2 matmul ----------------
    xgT = []
    for k in range(8):
        ps = ppT.tile([128, 128], F32, space="PSUM", tag="pt")
        nc.tensor.transpose(out=ps[:], in_=gath[:, k * 128:(k + 1) * 128],
                            identity=id_sb[:])
        t = p2s.tile([128, 128], BF16, tag=f"xgT{k}")
        nc.scalar.activation(t[:], ps[:], AF.Copy)
        xgT.append(t)
    x9 = p2s.tile([128, 128], BF16, tag="x9")
    nc.vector.memset(x9[:], 0.0)
    nc.vector.memset(x9[0:1, :], 1.0)
    xgT.append(x9)

    pgc = p2s.tile([128, 1], F32, tag="pgc")
    nc.vector.tensor_copy(out=pgc[:], in_=gath[:, H2 + L:H2 + L + 1])
    nc.sync.dma_start(out=o_pg[:], in_=pgc[:])
    vg_bf = p2s.tile([128, L], BF16, tag="vg_bf")
    nc.vector.tensor_copy(out=vg_bf[:], in_=gath[:, H2:H2 + L])

    # scatter passes
    proj = p2s.tile([128, WF], BF16, tag="proj")
    projx = p2s.tile([128, WF], BF16, tag="projx")
    psx_cm = tc.tile_pool(name="psx", bufs=2)
    psx = psx_cm.__enter__()
    for p in range(NPASS):
        tgt = proj if p == 0 else projx
        for s in range(NSS):
            it = psx.tile([128, L], I16, tag="sidx")
            nc.sync.dma_start(out=it[:], in_=sidx[p, s])
            nc.gpsimd.local_scatter(out_ap=tgt[:, s * SS:(s + 1) * SS],
                                    data_ap=vg_bf[:], idxs_ap=it[:],
                                    channels=128, num_elems=SS, num_idxs=L)
        if p > 0:
            nc.vector.tensor_add(out=proj[:], in0=proj[:], in1=projx[:])
    psx_cm.__exit__(None, None, None)

    # V2 matmul + vocab softmax partials
    v2sb = [rtile([128, VSP], BF16, f"v2sb{k}", v2T[k], pool=p2s) for k in range(KV2)]
    expl = p2s.tile([128, VSP], BF16, tag="expl")
    zvp = p2s.tile([128, len(CHUNKS_V)], F32, tag="zvp")
    for ci, (off, w) in enumerate(CHUNKS_V):
        ps = ppB.tile([128, 512], F32, space="PSUM")
        for k in range(KV2):
            nc.tensor.matmul(ps[:, :w], lhsT=xgT[k][:], rhs=v2sb[k][:, off:off + w],
                             start=(k == 0), stop=(k == KV2 - 1))
        nc.scalar.activation(expl[:, off:off + w], ps[:, :w], AF.Exp,
                             accum_out=zvp[:, ci:ci + 1])
    zv_part = p2s.tile([128, 1], F32, tag="zv_part")
    nc.vector.reduce_sum(out=zv_part[:], in_=zvp[:], axis=mybir.AxisListType.X)
    nc.sync.dma_start(out=zv_in[:], in_=zv_part[:])
    nc.gpsimd.collective_compute("AllReduce", OP.add, replica_groups=RG,
                                 ins=[zv_in[:]], outs=[zv_out[:]])
    zv_sb = p2s.tile([128, 1], F32, tag="zv_sb")
    nc.sync.dma_start(out=zv_sb[:], in_=zv_out[:])
    rzv = p2s.tile([128, 1], F32, tag="rzv")
    nc.vector.reciprocal(out=rzv[:], in_=zv_sb[:])
    vsc = p2s.tile([128, 1], F32, tag="vsc")
    nc.vector.tensor_mul(out=vsc[:], in0=rzv[:], in1=pgc[:])

    # vocab_ext, fin = exp(vocab_ext + proj), final = fin / Z2
    fexp = p2s.tile([128, WF], BF16, tag="fexp")
    z2p = p2s.tile([128, len(CHUNKS_ALL)], F32, tag="z2p")
    pvx_cm = tc.tile_pool(name="pvx", bufs=3)
    pvx = pvx_cm.__enter__()
    for ci, (off, w) in enumerate(CHUNKS_V):
        vx = pvx.tile([128, 512], F32, tag="vx")
        nc.vector.tensor_scalar_mul(out=vx[:, :w], in0=expl[:, off:off + w],
                                    scalar1=vsc[:])
        nc.sync.dma_start(out=o_vext[:, off:off + w], in_=vx[:, :w])
        fp = pvx.tile([128, 512], F32, tag="fp")
        nc.vector.tensor_add(out=fp[:, :w], in0=vx[:, :w],
                             in1=proj[:, off:off + w])
        nc.scalar.activation(fexp[:, off:off + w], fp[:, :w], AF.Exp,
                             accum_out=z2p[:, ci:ci + 1])
    for cj, (off, w) in enumerate(CHUNKS_U):
        ci = len(CHUNKS_V) + cj
        nc.scalar.activation(fexp[:, off:off + w], proj[:, off:off + w], AF.Exp,
                             accum_out=z2p[:, ci:ci + 1])
    pvx_cm.__exit__(None, None, None)

    z2_part = p2s.tile([128, 1], F32, tag="z2_part")
    nc.vector.reduce_sum(out=z2_part[:], in_=z2p[:], axis=mybir.AxisListType.X)
    z2_adj = p2s.tile([128, 1], F32, tag="z2_adj")
    nc.vector.tensor_scalar_add(out=z2_adj[:], in0=z2_part[:],
                                scalar1=float(-DEAD_COLS))
    nc.sync.dma_start(out=z2_in[:], in_=z2_adj[:])
    nc.gpsimd.collective_compute("AllReduce", OP.add, replica_groups=RG,
                                 ins=[z2_in[:]], outs=[z2_out[:]])
    z2_sb = p2s.tile([128, 1], F32, tag="z2_sb")
    nc.sync.dma_start(out=z2_sb[:], in_=z2_out[:])
    rz2 = p2s.tile([128, 1], F32, tag="rz2")
    nc.vector.reciprocal(out=rz2[:], in_=z2_sb[:])

    pfo_cm = tc.tile_pool(name="pfo", bufs=3)
    pfo = pfo_cm.__enter__()
    for off, w in CHUNKS_ALL:
        fo = pfo.tile([128, 512], F32, tag="fo")
        nc.vector.tensor_scalar_mul(out=fo[:, :w], in0=fexp[:, off:off + w],
                                    scalar1=rz2[:])
        nc.sync.dma_start(out=o_final[:, off:off + w], in_=fo[:, :w])
    pfo_cm.__exit__(None, None, None)

    p2s_cm.__exit__(None, None, None)
    res_cm.__exit__(None, None, None)
    ppB_cm.__exit__(None, None, None)
    pcol_cm.__exit__(None, None, None)
    ppT_cm.__exit__(None, None, None)
    ppS_cm.__exit__(None, None, None)
    tc_cm.__exit__(None, None, None)
    nc.compile()
    return nc


# ----------------------------------------------------------------- host prep
def _prep_weights(params):
    p = {k: np.asarray(v, dtype=np.float32) if not isinstance(v, dict) else v
         for k, v in params.items()}

    def col(x):
        return _f32(x).reshape(-1, 1)

    w = {}
    w["whtT"] = _bf(p["Wh_t_w"].T)
    w["wstT"] = _bf(p["Ws_t_w"].T)
    w["vt"] = _bf(p["v_t_w"].reshape(-1, 1))
    w["biast"] = col(p["Wh_t_b"] + p["Ws_t_b"])
    w["whaT"] = _bf(p["Wh_a_w"].T)
    w["wsaT"] = _bf(p["Ws_a_w"].T)
    w["va"] = _bf(p["v_a_w"].reshape(-1, 1))
    w["biasa"] = col(p["Wh_a_b"] + p["Ws_a_b"])

    def gru_w(gp, pre):
        wih = np.asarray(gp["Wih"], np.float32)
        whh = np.asarray(gp["Whh"], np.float32)
        bih = np.asarray(gp["bih"], np.float32)
        bhh = np.asarray(gp["bhh"], np.float32)
        h = whh.shape[1]
        return {
            f"{pre}_wihT": _bf(wih.T),
            f"{pre}_whhT": _bf(whh.T),
            f"{pre}_brz": col(bih[:2 * h] + bhh[:2 * h]),
            f"{pre}_bin": col(bih[2 * h:]),
            f"{pre}_bhn": col(bhh[2 * h:]),
        }

    w.update(gru_w(params["gru"], "gd"))
    w.update(gru_w(params["gruv_f"], "gvf"))
    w.update(gru_w(params["gruv_b"], "gvb"))

    w["wtT"] = _bf(p["wt_w"].T)
    w["ws2T"] = _bf(p["ws2_w"].T)
    w["v2c"] = _bf(p["v_w"].reshape(-1, 1))
    w["bias2"] = col(p["wt_b"] + p["ws2_b"])
    w["whc"] = _bf(p["wh_w"].reshape(-1, 1))
    w["wsc"] = _bf(p["ws_w"].reshape(-1, 1))
    w["wxc"] = _bf(p["wx_w"].reshape(-1, 1))
    w["pgb"] = np.full((128, 1), float(np.asarray(p["wh_b"]).ravel()[0] + np.asarray(p["ws_b"]).ravel()[0] + np.asarray(p["wx_b"]).ravel()[0]), np.float32)
    w["v1T"] = _bf(p["V1_w"].T)
    w["v1b"] = col(p["V1_b"])
    w["ident"] = np.eye(128, dtype=np.float32)
    return w


def _prep_v2(params):
    """Per-core padded V2^T with bias row: [KV2, 128, VSP] bf16."""
    v2w = np.asarray(params["V2_w"], np.float32)   # [50000, 1024]
    v2b = np.asarray(params["V2_b"], np.float32)   # [50000]
    out = []
    for c in range(NCORE):
        sl = slice(c * VS, (c + 1) * VS)
        blk = np.zeros((KV2 * 128, VSP), np.float32)
        blk[:H2, :VS] = v2w[sl].T
        blk[H2, :VS] = v2b[sl]
        blk[H2, VS:] = -30.0     # dead cols get exp(-30) ~ 0 in vocab softmax
        out.append(_bf(blk).reshape(KV2, 128, VSP))
    return out


def _prep_sidx(data):
    """Per-core remapped scatter index arrays [NPASS, NSS, 128, 640] int16."""
    d = np.asarray(data).astype(np.int64)  # [128, 640]
    owner = np.where(d < VOCAB, d // VS, (d - VOCAB) // UP)
    local = np.where(d < VOCAB, d - owner * VS, VSP + (d - VOCAB) - owner * UP)
    out = []
    dropped = 0
    for c in range(NCORE):
        idx = np.full((NPASS, NSS, 128, L), -1, np.int16)
        mine = owner == c
        for b in range(B):
            js = np.nonzero(mine[b])[0]
            if len(js) == 0:
                continue
            locs = local[b, js]
            order = np.argsort(locs, kind="stable")
            sl = locs[order]
            first = np.r_[True, sl[1:] != sl[:-1]]
            grp = np.cumsum(first) - 1
            starts = np.nonzero(first)[0]
            occ_sorted = np.arange(len(sl)) - starts[grp]
            occ = np.empty(len(js), np.int64)
            occ[order] = occ_sorted
            for j, lc, oc in zip(js, locs, occ):
                if oc >= NPASS:
                    dropped += 1
                    continue
                s = int(lc) // SS
                idx[oc, s, b, j] = np.int16(lc - s * SS)
        out.append(idx)
    if dropped:
        print(f"[kernel] warning: dropped {dropped} scatter dup entries (>={NPASS}x)")
    return out


def prep_in_maps(data, target_input, words_padding_mask, hidden, outputs_t,
                 outputs_a, hidden_c, params):
    tgt = _f32(target_input)[0]          # [128, 256]
    hid = _f32(hidden)[0]                # [128, 512]
    hc = _f32(hidden_c)                  # [2, 128, 512]
    mask = _f32(words_padding_mask)      # [128, 640]
    ot = np.asarray(outputs_t, np.float32)   # [512, 128, 1024]
    oa = np.asarray(outputs_a, np.float32)   # [128, 128, 1024]

    w = _prep_weights(params)
    v2_per_core = _prep_v2(params)
    sidx_per_core = _prep_sidx(data)

    in_maps = []
    for c in range(NCORE):
        rc = slice(c * BC, (c + 1) * BC)
        otc = ot[:, rc, :]               # [512, 16, 1024]
        oac = oa[:, rc, :]               # [128, 16, 1024]
        otT = _bf(otc.transpose(2, 1, 0))     # [1024, 16, 512]
        ot4 = np.ascontiguousarray(
            otT.reshape(8, 128, 4, 4, 512).transpose(0, 2, 1, 3, 4)
        ).reshape(8, 4, 128, 2048)
        otn = np.ascontiguousarray(
            _bf(otc.transpose(1, 0, 2)).reshape(16, 4, 128, 1024))
        oaT = _bf(oac.transpose(2, 1, 0))     # [1024, 16, 128]
        oa8 = np.ascontiguousarray(oaT.reshape(8, 128, 16, 128)).reshape(8, 128, 2048)
        oan = np.ascontiguousarray(_bf(oac.transpose(1, 0, 2)))
        m = dict(w)
        m.update(
            ot4=ot4, otn=otn, oa8=oa8, oan=oan,
            xT=_f32(tgt[rc].T), hT=_f32(hid[rc].T),
            hcfT=_f32(hc[0, rc].T), hcbT=_f32(hc[1, rc].T),
            maskd=_f32(mask[rc]),
            sidx=sidx_per_core[c], v2T=v2_per_core[c],
        )
        in_maps.append(m)
    return in_maps


_NC_CACHE = None


def get_nc():
    global _NC_CACHE
    if _NC_CACHE is None:
        _NC_CACHE = build_nc()
    return _NC_CACHE


def assemble(results):
    S = np.concatenate([results[c]["S"] for c in range(NCORE)], axis=0)
    attn = np.concatenate([results[c]["attn"] for c in range(NCORE)], axis=0)
    pg = results[0]["pg"]
    final = np.zeros((B, EXT), np.float32)
    vext = np.zeros((B, EXT), np.float32)
    for c in range(NCORE):
        f = results[c]["final"]
        final[:, c * VS:(c + 1) * VS] = f[:, :VS]
        final[:, VOCAB + c * UP:VOCAB + (c + 1) * UP] = f[:, VSP:VSP + UP]
        vext[:, c * VS:(c + 1) * VS] = results[c]["vext"][:, :VS]
    return S, attn, pg, final, vext


def kernel(**inputs):
    nc = get_nc()
    in_maps = prep_in_maps(**inputs)
    res = run_bass_kernel_spmd(nc, in_maps, list(range(NCORE)))
    return assemble(res.results)


if __name__ == "__main__":
    nc = build_nc()
    print("built ok")
